# revision 45
# baseline (speedup 1.0000x reference)
"""Trainium2 Bass kernel for nn_Action_Decoder (GAT-based action decoder).

Strategy (8 NeuronCores, pure data-parallel over batch):
  - B=4096 sharded 8 x 512 samples/core; all weights replicated.
  - Host pre-gathers the 7 needed rows per sample (6 config nodes + the
    chosen substation) and pre-transposes each 128-sample tile to
    [channel, (k,b)] bf16, so the device reads ~1MB instead of the full
    28MB embedding tables and needs no on-device gather.
  - Layer-1 GAT: h^T = W1^T @ [obs_repr | sub | node] via PE matmuls;
    e_src/e_dst via PE matmuls against per-head selector matrices; h back
    to batch layout via PE transposes (hbL blocks in (j,h,f) order).
  - Hidden channels are host-permuted from (head, feat) to (feat, head)
    order so every attention-apply operand has a step-1 bf16 4-elem head
    run innermost: the alpha broadcast rides a stride-0 MIDDLE dim and
    the DVE still engages 2x packed mode (mode detect checks only the
    last AP dim). The j-fold is 3 big contiguous adds over all i (2x).
  - elu (exact: exp(min(x,0)) + relu(x) - 1, the -1 folded into sum(W2));
    layer-2's W2 contraction is one 2x DVE product + per-i sums on the
    scalar engine's activation accumulator (split scalar/vector on the
    last tile to shorten the kernel tail); tiny batched 6x6 attention
    on vector/scalar.
  - Inputs ship as 4 consolidated DMAs (obs_T, gathered table, one bf16
    pack, one f32 pack) since input bytes + per-DMA sequencer issue cost
    dominate kernel startup.
"""

import os
import sys

import numpy as np

for _p in ("/root/.axon_site", "/root/.axon_site/_ro/trn_rl_repo",
           "/root/.axon_site/_ro/pypackages", "/opt/trn_rl_repo", "/opt/pypackages"):
    if os.path.isdir(_p) and _p not in sys.path:
        sys.path.append(_p)

import ml_dtypes

# If this container lacks the antenv.axon_hooks NTFF-profiling shim,
# inject a no-op registry so run_bass_kernel_spmd's trace path degrades
# to "hook isn't registered" instead of crashing kernel() under
# BASS_TRACE=1. No effect when the real module exists.
try:
    import antenv.axon_hooks  # noqa: F401
except ImportError:
    import types

    import antenv

    _m = types.ModuleType("antenv.axon_hooks")
    _m._HOOK = None

    def _set_hook(hook, _m=_m):
        _m._HOOK = hook

    def _get_hook(_m=_m):
        return _m._HOOK

    _m.set_axon_ntff_profile_hook = _set_hook
    _m.get_axon_ntff_profile_hook = _get_hook
    sys.modules["antenv.axon_hooks"] = _m
    antenv.axon_hooks = _m

import concourse.bass as bass
import concourse.tile as tile
from concourse import bacc
from concourse import mybir
from concourse.bass_utils import run_bass_kernel_spmd

# Problem dims
B, N, S, K, H, OBS = 4096, 177, 36, 6, 128, 500
HEADS, FH = 4, 32
C_IN = 3 * H
NCORES = 8
BS = B // NCORES          # 512 samples per core
NT = BS // 128            # 4 tiles of 128 samples
OBS_PAD = 512             # pad 500 -> 512
R = N + S                 # combined table rows per sample (213)

F32 = mybir.dt.float32
BF16 = mybir.dt.bfloat16
I16 = mybir.dt.int16
AX = mybir.AxisListType
OP = mybir.AluOpType
ACT = mybir.ActivationFunctionType

LRELU_SLOPE = 0.2


def build_graph(scalars):
    as2 = float(scalars["a_src2"])
    ad2 = float(scalars["a_dst2"])
    b2 = float(scalars["b2"])
    c2 = float(scalars["c2"])

    nc = bacc.Bacc(num_swdge_queues=4)

    # packed params: one bf16 pack [128, 1160] = wproj(4x128) | w1(3x128) |
    # asrc(8) | ident(128) | w2rep(128); one f32 pack [128, 50] =
    # bproj(1) | b1(1) | ce_rep(48)  -- 2 const DMAs instead of 9
    obs_T = nc.declare_dram_parameter("obs_T", [OBS_PAD, BS], BF16, isOutput=False)
    gth_d = nc.declare_dram_parameter("gth_d", [NT * 128, 896], BF16, isOutput=False)
    bfpack = nc.declare_dram_parameter("bfpack", [128, 1160], BF16, isOutput=False)
    fpack = nc.declare_dram_parameter("fpack", [128, 50], F32, isOutput=False)
    out_ext = nc.declare_dram_parameter("out", [BS, K], F32, isOutput=True)

    with tile.TileContext(nc) as tc:
        with (
            tc.tile_pool(name="consts", bufs=1) as consts,
            tc.tile_pool(name="obsp", bufs=1) as obsp,
            tc.tile_pool(name="gat", bufs=4) as gat,
            tc.tile_pool(name="work", bufs=4) as work,
            tc.tile_pool(name="big", bufs=2) as big,
            tc.tile_pool(name="small", bufs=3) as small,
            tc.tile_pool(name="psA", bufs=2, space="PSUM") as psA,
            tc.tile_pool(name="psB", bufs=2, space="PSUM") as psB,
            tc.tile_pool(name="psC", bufs=2, space="PSUM") as psC,
        ):
            # ---- consolidated input DMAs: bf16 pack + 2x gth halves on
            # the scalar queue; obs + f32 pack on sync ----
            pack_sb = consts.tile([128, 1160], BF16)
            nc.scalar.dma_start(out=pack_sb[:, :], in_=bfpack[:, :])
            gth_all = consts.tile([128, NT, 896], BF16)
            for half in range(2):
                nc.scalar.dma_start(
                    out=gth_all[:, 2 * half:2 * half + 2, :],
                    in_=bass.AP(tensor=gth_d, offset=half * 2 * 128 * 896,
                                ap=[[896, 128], [128 * 896, 2], [1, 896]]),
                )
            obs_in = obsp.tile([128, 4, BS], BF16)
            nc.sync.dma_start(
                out=obs_in[:, :, :],
                in_=bass.AP(tensor=obs_T, offset=0,
                            ap=[[BS, 128], [128 * BS, 4], [1, BS]]),
            )
            fp_sb = consts.tile([128, 50], F32)
            nc.sync.dma_start(out=fp_sb[:, :], in_=fpack[:, :])

            wproj_sb = pack_sb[:, 0:512].rearrange("p (c x) -> p c x", c=4)
            w1_sb = pack_sb[:, 512:896].rearrange("p (c x) -> p c x", c=3)
            asrc_sb = pack_sb[:, 896:904]
            ident = pack_sb[:, 904:1032]
            w2_sb = pack_sb[:, 1032:1160]
            bproj_sb = fp_sb[:, 0:1]
            b1_sb = fp_sb[:, 1:2]
            ce_sb = fp_sb[:, 2:50]
            obs_ps = psA.tile([128, BS], F32, tag="hps")
            for c in range(4):
                nc.tensor.matmul(
                    obs_ps[:, :], wproj_sb[:, c, :], obs_in[:, c, :],
                    start=(c == 0), stop=(c == 3),
                )
            obs_sb = obsp.tile([128, BS], BF16)
            nc.scalar.activation(obs_sb[:, :], obs_ps[:, :], ACT.Identity,
                                 bias=bproj_sb, scale=1.0)

            # ---- per-tile pipeline ----
            for t in range(NT):
                gth = gth_all[:, t, :]

                # h^T = W1^T @ x^T : PSUM [128 (h,f), 768 (k,b)]
                # (obs contribution last: the sub/node matmuls start as
                # soon as the gather lands, without waiting on obs_sb)
                h_ps = psA.tile([128, 768], F32, tag="hps")
                for hs, nk in ((slice(0, 512), 4), (slice(512, 768), 2)):
                    rhs_obs = bass.AP(
                        tensor=obs_sb.tensor,
                        offset=obs_sb[:, :].offset + t * 128,
                        ap=[list(obs_sb[:, :].ap[0]), [0, nk], [1, 128]],
                    )
                    rhs_sub = bass.AP(
                        tensor=gth.tensor, offset=gth.offset + 512,
                        ap=[list(gth.ap[0]), [0, nk], [1, 128]],
                    )
                    nc.tensor.matmul(h_ps[:, hs], w1_sb[:, 1, :], rhs_sub,
                                     start=True, stop=False)
                    node_rhs = (gth_all[:, t, 0:512] if hs.start == 0
                                else gth_all[:, t, 640:896])
                    nc.tensor.matmul(h_ps[:, hs], w1_sb[:, 2, :], node_rhs,
                                     start=False, stop=False)
                    nc.tensor.matmul(h_ps[:, hs], w1_sb[:, 0, :], rhs_obs,
                                     start=False, stop=True)

                # h'^T (+b1) -> SBUF bf16
                hT_sb = work.tile([128, 768], BF16)
                nc.scalar.activation(hT_sb[:, :], h_ps[:, :], ACT.Identity,
                                     bias=b1_sb, scale=1.0)

                # e_src/e_dst per k-slice: PSUM [128b, 48=(k,sd,h)]
                e_ps = psB.tile([128, 48], F32)
                for k in range(K):
                    nc.tensor.matmul(
                        e_ps[:, k * 8:(k + 1) * 8],
                        hT_sb[:, k * 128:(k + 1) * 128], asrc_sb,
                        start=True, stop=True,
                    )
                e_sd = small.tile([128, 48], F32)
                nc.vector.tensor_sub(e_sd[:, :], e_ps[:, :], ce_sb)

                # h' to batch layout, strided so hbL is [128b, (h,f,j)]
                hbL_ps = psC.tile([128, 768], BF16)
                for k in range(K):
                    nc.tensor.transpose(
                        hbL_ps[:, k * 128:(k + 1) * 128],
                        hT_sb[:, k * 128:(k + 1) * 128], ident,
                    )
                hbL = work.tile([128, 768], BF16)
                nc.scalar.activation(hbL[:, :], hbL_ps[:, :], ACT.Copy)

                # ---- alpha phase (batch layout, (i,j,h) order) ----
                e_sd4 = e_sd[:, :].rearrange("p (k s h) -> p k s h", s=2, h=4)
                e_dst_ap = (
                    e_sd4[:, :, 1, :].unsqueeze(2).broadcast_to([128, 6, 6, 4])
                )
                e_src_ap = (
                    e_sd4[:, :, 0, :].unsqueeze(1).broadcast_to([128, 6, 6, 4])
                )
                e_raw = small.tile([128, 144], F32)
                nc.vector.tensor_tensor(
                    e_raw[:, :].rearrange("p (i j h) -> p i j h", j=6, h=4),
                    e_dst_ap, e_src_ap, OP.add,
                )
                e_lr = small.tile([128, 144], F32)
                nc.vector.scalar_tensor_tensor(
                    e_lr[:, :], e_raw[:, :], LRELU_SLOPE, e_raw[:, :],
                    OP.mult, OP.max)
                E1 = small.tile([128, 144], F32)
                nc.scalar.activation(E1[:, :], e_lr[:, :], ACT.Exp)
                E1v = (E1[:, :].rearrange("p (i j h) -> p i j h", j=6, h=4)
                       .transpose([0, 1, 3, 2]))       # [p, i, h, j]
                Z1 = small.tile([128, 24], F32)
                nc.vector.tensor_reduce(
                    Z1[:, :].rearrange("p (i h) -> p i h", i=6),
                    E1v, axis=AX.X, op=OP.add)
                rZ1 = small.tile([128, 24], F32)
                nc.vector.reciprocal_approx_fast(rZ1[:, :], Z1[:, :])
                al = small.tile([128, 144], BF16)
                nc.vector.tensor_mul(
                    al[:, :].rearrange("p (i j h) -> p i j h", j=6, h=4),
                    E1[:, :].rearrange("p (i j h) -> p i j h", j=6, h=4),
                    rZ1[:, :].rearrange("p (i h) -> p i h", i=6)
                    .unsqueeze(2).broadcast_to([128, 6, 6, 4]),
                )
                # ---- attention apply: prod[(i),f,j,h] = al * hbL. The f
                # broadcast sits on a stride-0 MIDDLE dim; the innermost
                # dim is a step-1 bf16 4-elem head run on every operand,
                # which is what the DVE needs for 2x packed mode.
                alv = al[:, :].rearrange("p (i j h) -> p i j h", j=6, h=4)
                hv = (hbL[:, :].rearrange("p (j f h) -> p j f h", f=32, h=4)
                      .transpose([0, 2, 1, 3]))        # [p, f, j, h]
                prod_all = work.tile([128, 4608], BF16, tag="prod")
                for i in range(K):
                    nc.vector.tensor_mul(
                        prod_all[:, i * 768:(i + 1) * 768].rearrange(
                            "p (f j h) -> p f j h", j=6, h=4),
                        alv[:, i, :, :].unsqueeze(1)
                        .broadcast_to([128, 32, 6, 4]),
                        hv,
                    )
                pa = prod_all[:, :].rearrange("p (x j h) -> p x j h",
                                              j=6, h=4)
                a1 = work.tile([128, 2304], BF16, tag="fs")
                av = a1[:, :].rearrange("p (x j h) -> p x j h", j=3, h=4)
                nc.vector.tensor_add(av, pa[:, :, 0:3, :], pa[:, :, 3:6, :])
                ft = work.tile([128, 768], BF16, tag="ft")
                ftv = ft[:, :].rearrange("p (x h) -> p x h", h=4)
                nc.vector.tensor_add(ftv, av[:, :, 0, :], av[:, :, 1, :])
                attn = work.tile([128, 768], BF16)  # layout (i, f, h)
                nc.vector.tensor_add(
                    attn[:, :].rearrange("p (x h) -> p x h", h=4),
                    ftv, av[:, :, 2, :])

                # ---- elu(x) = exp(min(x,0)) + relu(x) - 1 (-1 folds into c2)
                min_x = work.tile([128, 768], BF16)
                nc.vector.tensor_scalar_min(min_x[:, :], attn[:, :], 0.0)
                exp_m = work.tile([128, 768], BF16)
                nc.scalar.activation(exp_m[:, :], min_x[:, :], ACT.Exp)
                # relu fused into the sum: v1 = max(attn,0) + exp_m (stt
                # runs 2x like the add it replaces; drops a scalar op +
                # a serial chain hop)
                v1 = work.tile([128, 768], BF16)
                nc.vector.scalar_tensor_tensor(
                    v1[:, :], attn[:, :], 0.0, exp_m[:, :],
                    OP.max, OP.add)

                # ---- layer 2: h2 = sum_hf (v-1)*W2; one 2x DVE product,
                # then the six per-i sums ride the scalar engine's
                # activation accumulator instead of DVE affine_mul_reduce
                y2 = work.tile([128, 768], BF16, tag="y2")
                nc.vector.tensor_mul(
                    y2[:, :].rearrange("p (i x) -> p i x", i=6),
                    v1[:, :].rearrange("p (i x) -> p i x", i=6),
                    w2_sb.unsqueeze(1).broadcast_to([128, 6, 128]),
                )
                h2 = small.tile([128, 6], F32)
                scr = work.tile([128, 128], BF16, tag="scr")
                # last tile: split the six sums across scalar+vector so the
                # kernel's tail chain is ~2x shorter
                n_sc = K if t < NT - 1 else K // 2
                for i in range(n_sc):
                    nc.scalar.activation(
                        scr[:, :], y2[:, i * 128:(i + 1) * 128], ACT.Copy,
                        accum_out=h2[:, i:i + 1],
                    )
                if n_sc < K:
                    scrv = work.tile([128, 128], F32, tag="scrv")
                    for i in range(n_sc, K):
                        nc.vector.affine_mul_reduce(
                            out=scrv[:, :], accum_out=h2[:, i:i + 1],
                            in0=v1[:, i * 128:(i + 1) * 128], in1=w2_sb,
                            scale=1.0, bias=0.0,
                        )
                h2c = small.tile([128, 6], F32)
                nc.vector.tensor_scalar(h2c[:, :], h2[:, :], -c2, None, OP.add)
                h2s = small.tile([128, 6], F32)
                nc.vector.tensor_scalar(h2s[:, :], h2[:, :], as2,
                                        -c2 * (as2 + ad2), OP.mult, OP.add)
                # e2 = ad2*h2[i] + (as2*h2[j] - c2*(as2+ad2))
                e2_raw = small.tile([128, 36], F32)
                nc.vector.scalar_tensor_tensor(
                    e2_raw[:, :].rearrange("p (i j) -> p i j", j=6),
                    h2[:, :].unsqueeze(2).broadcast_to([128, 6, 6]),
                    ad2,
                    h2s[:, :].unsqueeze(1).broadcast_to([128, 6, 6]),
                    OP.mult, OP.add,
                )
                e2_lr = small.tile([128, 36], F32)
                nc.vector.scalar_tensor_tensor(
                    e2_lr[:, :], e2_raw[:, :], LRELU_SLOPE, e2_raw[:, :],
                    OP.mult, OP.max)
                E2 = small.tile([128, 36], F32)
                nc.scalar.activation(E2[:, :], e2_lr[:, :], ACT.Exp)
                E2v = E2[:, :].rearrange("p (i j) -> p i j", j=6)
                Z2 = small.tile([128, 6], F32)
                nc.vector.tensor_reduce(Z2[:, :], E2v, axis=AX.X, op=OP.add)
                rZ2 = small.tile([128, 6], F32)
                nc.vector.reciprocal_approx_fast(rZ2[:, :], Z2[:, :])
                P2 = small.tile([128, 36], F32)
                nc.vector.tensor_mul(
                    P2[:, :].rearrange("p (i j) -> p i j", j=6),
                    E2v,
                    h2c[:, :].unsqueeze(1).broadcast_to([128, 6, 6]),
                )
                S2 = small.tile([128, 6], F32)
                nc.vector.tensor_reduce(
                    S2[:, :], P2[:, :].rearrange("p (i j) -> p i j", j=6),
                    axis=AX.X, op=OP.add,
                )
                out_sb = small.tile([128, 6], F32)
                nc.vector.tensor_mul(out_sb[:, :], S2[:, :], rZ2[:, :])
                if b2 != 0.0:
                    nc.vector.tensor_scalar(out_sb[:, :], out_sb[:, :], b2,
                                            None, OP.add)
                nc.sync.dma_start(out=out_ext[t * 128:(t + 1) * 128, :],
                                  in_=out_sb[:, :])

    nc.finalize()
    return nc


def prep_core_inputs(core, org_obs, node_embeddings, substation_embeddings,
                     sub_choice, sub_id_to_elem_id, W_proj, b_proj, W1,
                     a_src1, a_dst1, b1, W2, a_src2, a_dst2, b2):
    """Host-side shard + layout prep for one core (index math and weight
    folding only -- all tensor FLOPs stay on device)."""
    bf = ml_dtypes.bfloat16
    s = slice(core * BS, (core + 1) * BS)

    obs = org_obs[s]
    obs_T = np.zeros((OBS_PAD, BS), np.float32)
    obs_T[:OBS, :] = obs.T
    obs_T = obs_T.astype(bf)

    # host pre-gather: 7 rows per sample [node k0..3 | sub | node k4,k5],
    # pre-transposed per tile to [chan(128), (k,b)] for the TensorEngine
    sub_idx = sub_choice[s, 0].astype(np.int64)
    elem = sub_id_to_elem_id[sub_idx]              # [BS, K]
    node = np.asarray(node_embeddings[s], np.float32)
    subs = np.asarray(substation_embeddings[s], np.float32)
    bloc = np.arange(BS)
    rows = np.empty((BS, 7, H), np.float32)
    rows[:, 0:4] = node[bloc[:, None], elem[:, 0:4]]
    rows[:, 4] = subs[bloc, sub_idx]
    rows[:, 5:7] = node[bloc[:, None], elem[:, 4:6]]
    gth_host = (rows.reshape(NT, 128, 7, H)
                .transpose(0, 3, 2, 1)            # [NT, chan, k, b]
                .reshape(NT * 128, 7 * 128).astype(bf))

    wp = np.zeros((OBS_PAD, H), np.float32)
    wp[:OBS] = W_proj
    # hidden channels permuted (h,f) -> (f,h) so the apply's innermost
    # dim is the 4 heads (step-1 operands everywhere -> DVE 2x mode)
    perm = np.array([(q % HEADS) * FH + q // HEADS for q in range(H)])
    # bf16 pack [128, 1160]: wproj | w1 | asrc | ident | w2rep, laid out so
    # pack[p, c*128+x] = w[c*128+p, x] for the chunked weights
    wp_t = wp.reshape(4, 128, H).transpose(1, 0, 2).reshape(128, 512)
    w1_t = np.asarray(W1, np.float32)[:, perm].reshape(3, 128, H) \
        .transpose(1, 0, 2).reshape(128, 384)
    bfpack = np.concatenate([
        wp_t, w1_t, _asrc_mat(a_src1, a_dst1)[perm, :],
        np.eye(128, dtype=np.float32),
        np.tile(np.asarray(W2, np.float32).reshape(1, H)[:, perm], (128, 1)),
    ], axis=1).astype(bf)
    fpack = np.concatenate([
        np.asarray(b_proj, np.float32).reshape(H, 1),
        np.asarray(b1, np.float32).reshape(H, 1)[perm],
        np.tile(_ce_const(b1, a_src1, a_dst1).reshape(1, 48), (H, 1)),
    ], axis=1).astype(np.float32)
    return {
        "obs_T": obs_T,
        "gth_d": gth_host,
        "bfpack": bfpack,
        "fpack": fpack,
    }


def _asrc_mat(a_src1, a_dst1):
    m = np.zeros((H, 8), np.float32)
    for h in range(HEADS):
        m[h * FH:(h + 1) * FH, h] = a_src1[h]
        m[h * FH:(h + 1) * FH, 4 + h] = a_dst1[h]
    return m


def _ce_const(b1, a_src1, a_dst1):
    c = np.zeros((K, 2, HEADS), np.float32)
    b1r = np.asarray(b1, np.float32).reshape(HEADS, FH)
    c[:, 0, :] = (b1r * a_src1).sum(-1)[None, :]
    c[:, 1, :] = (b1r * a_dst1).sum(-1)[None, :]
    return c


_GRAPH_CACHE = {}
LAST_RESULTS = None


def kernel(**inputs):
    inp = {k: np.asarray(v) for k, v in inputs.items()}
    W2 = np.asarray(inp["W2"], np.float32)
    scalars = {
        "a_src2": float(np.asarray(inp["a_src2"]).reshape(-1)[0]),
        "a_dst2": float(np.asarray(inp["a_dst2"]).reshape(-1)[0]),
        "b2": float(np.asarray(inp["b2"]).reshape(-1)[0]),
        "c2": float(W2.sum()),
    }
    key = tuple(sorted(scalars.items()))
    if key not in _GRAPH_CACHE:
        _GRAPH_CACHE[key] = build_graph(scalars)
    nc = _GRAPH_CACHE[key]

    in_maps = [
        prep_core_inputs(
            c, inp["org_obs"], inp["node_embeddings"],
            inp["substation_embeddings"], inp["sub_choice"],
            inp["sub_id_to_elem_id"], inp["W_proj"], inp["b_proj"], inp["W1"],
            inp["a_src1"], inp["a_dst1"], inp["b1"], inp["W2"], inp["a_src2"],
            inp["a_dst2"], inp["b2"],
        )
        for c in range(NCORES)
    ]
    res = run_bass_kernel_spmd(nc, in_maps, core_ids=list(range(NCORES)))
    global LAST_RESULTS
    LAST_RESULTS = res
    out = np.concatenate([res.results[c]["out"] for c in range(NCORES)], axis=0)
    return out.reshape(B, K, 1).astype(np.float32)


if __name__ == "__main__":
    g = build_graph({"a_src2": 0.01, "a_dst2": 0.02, "b2": 0.0, "c2": 0.1})
    print("graph built ok")

